# revision 25
# baseline (speedup 1.0000x reference)
"""Trainium2 Bass kernel for windowed/global sparse attention (Swin-style
relative-position bias + 1 global token), data-parallel over batch on 8 cores.

Shapes: B=16, N=785 (1 global + 28x28 local), C=768, H=12 heads, d=64.

v2 design notes (from trace analysis of v1, which was ACT-bound at 97.7%):
  - Head-PAIR-merged attention inner loop: both heads of a pair write one
    4-bank PSUM tile ([128, 2048] f32, hh1 at column 1024), so exp is a
    SINGLE ACTIVATE per (pair, kc) with a 3D access pattern (FD=1572),
    halving ScalarE instruction count and its ~470ns/instr overhead.
    The expB multiply likewise becomes a single DVE tensor_tensor on a
    host-pre-paired [128, 1572] bf16 tile.
  - S^T matmuls for the two heads run concurrently via PE row-tiling
    (tile_position auto-derived from base partitions 0/64, K=64 each).
  - Unified 12-slot pair pipeline across both batches with fillers
    (qkv chunks, V, x loads, proj of the previous batch) emitted BETWEEN
    pair emissions so the PSUM rotor interleaves them (v1's proj0 slid to
    the end of the program because its psum allocations sat behind all of
    batch-1's attention in rotor order, running ~55us at cold clock).
  - Denominators: ones-column in V (65th lhsT column) as v1; the
    reciprocal is computed on a [128, 74]-reshaped copy (DMA round trip
    through DRAM) instead of [12, 785] - DVE reciprocal is 8 cyc/elem
    per lane, so the reshape turns a 6.5us op into ~0.6us.
  - proj bias is added on the HOST (free), device emits bf16 output.
  - PSUM budget (8 banks): S-pair tile 4 + O convoy 2 + filler 2.
"""

import numpy as np
import ml_dtypes

import concourse.bass as bass
import concourse.bacc as bacc
import concourse.tile as tile
from concourse.tile import add_dep_helper
from concourse import mybir
from concourse.bass_utils import run_bass_kernel_spmd

F32 = mybir.dt.float32
BF16 = mybir.dt.bfloat16

WX = WY = 28
NGLO = 1
H = 12
L = WX * WY            # 784
N = NGLO + L           # 785
C = 768
HD = C // H            # 64
SCALE = HD ** -0.5
B = 16
N_CORES = 8
B_LOC = B // N_CORES   # 2
NCC = C // 128         # 6 contraction chunks
NKC = (N + 127) // 128  # 7 key/token chunks (last = 17 rows)
NPAIR = H // 2         # 6 head pairs
W = 786                # padded free width for N-sized tiles (even, 4B-aligned)
W2 = 2 * W             # pair-merged free width (1572)
SP_OFF = 1024          # hh1 column offset in the S-pair psum tile (bank 2)
DFLN = 74              # 128*74 = 9472 >= 12*785 reshaped reciprocal tile

CG_N = [(0, 512), (512, 274)]
CG_C = [(0, 512), (512, 256)]


def _kr(kc):
    return min(128, N - kc * 128)


def build_nc():
    nc = bacc.Bacc(None, target_bir_lowering=False)

    xT_d = nc.dram_tensor("xT", [B_LOC, C, N], BF16, kind="ExternalInput")
    qkvwT_d = nc.dram_tensor("qkv_wT", [C, 3 * C], BF16, kind="ExternalInput")
    pwT_d = nc.dram_tensor("proj_wT", [C, C], BF16, kind="ExternalInput")
    expP_d = nc.dram_tensor("expP", [NPAIR, N, W2], BF16, kind="ExternalInput")
    out_d = nc.dram_tensor("out", [B_LOC, N, C], BF16, kind="ExternalOutput")
    dall_d = nc.dram_tensor("dall_scratch", [B_LOC, 128 * DFLN], BF16)
    dinv_d = nc.dram_tensor("dinv_scratch", [B_LOC, 128 * DFLN], BF16)

    with tile.TileContext(nc) as tc:
        with (
            tc.tile_pool(name="consts", bufs=1) as consts,
            tc.tile_pool(name="perb", bufs=2) as perb,
            tc.tile_pool(name="expbp", bufs=6) as expbp,
            tc.tile_pool(name="flow", bufs=6) as flow,
            tc.tile_pool(name="ptp", bufs=20) as ptp,
            tc.tile_pool(name="norm", bufs=1) as norm,
            tc.tile_pool(name="outp", bufs=2) as outp,
            tc.tile_pool(name="pss", bufs=2, space=bass.MemorySpace.PSUM) as pss,
            tc.tile_pool(name="psm", bufs=2, space=bass.MemorySpace.PSUM) as psm,
        ):
            # ---- resident weights (bf16) ----
            qkvw = [consts.tile([128, 3 * C], BF16, tag=f"qkvw{cc}",
                                name=f"qkvw{cc}") for cc in range(NCC)]
            pw16 = [consts.tile([128, C], BF16, tag=f"pw{cc}",
                                name=f"pw{cc}") for cc in range(NCC)]

            def emit_weight_loads_proj():
                for cc in range(NCC):
                    nc.sync.dma_start(
                        pw16[cc][:], pwT_d[cc * 128:(cc + 1) * 128, :]
                    )

            def emit_x(b):
                xts = []
                for cc in range(NCC):
                    t = perb.tile([128, W], BF16, tag=f"xt{cc}", name=f"xt{cc}_{b}")
                    nc.sync.dma_start(
                        t[:, 0:N], xT_d[b, cc * 128:(cc + 1) * 128, :]
                    )
                    nc.vector.memset(t[:, N:W], 0.0)
                    xts.append(t)
                return xts

            def emit_x0_and_weights():
                # The q0/k0 weight column slices and x0 are prefetched first
                # (~1.6MB) so the first qkvT convoy -- and with it the whole
                # exp chain -- can start ~15us earlier than waiting for the
                # full 4.7MB weight+x load.
                xts[0] = []
                for cc in range(NCC):
                    t = perb.tile([128, W], BF16, tag=f"xt{cc}",
                                  name=f"xt{cc}_0")
                    nc.sync.dma_start(
                        qkvw[cc][:, 0:128], qkvwT_d[cc * 128:(cc + 1) * 128, 0:128]
                    )
                    nc.sync.dma_start(
                        qkvw[cc][:, C:C + 128],
                        qkvwT_d[cc * 128:(cc + 1) * 128, C:C + 128]
                    )
                    nc.sync.dma_start(
                        t[:, 0:N], xT_d[0, cc * 128:(cc + 1) * 128, :]
                    )
                    nc.vector.memset(t[:, N:W], 0.0)
                    xts[0].append(t)
                for cc in range(NCC):
                    nc.sync.dma_start(
                        qkvw[cc][:, 128:C], qkvwT_d[cc * 128:(cc + 1) * 128, 128:C]
                    )
                    nc.sync.dma_start(
                        qkvw[cc][:, C + 128:3 * C],
                        qkvwT_d[cc * 128:(cc + 1) * 128, C + 128:3 * C]
                    )

            # per-batch state
            xts = [None, None]
            qT = [[None] * NCC, [None] * NCC]
            kT = [[None] * NCC, [None] * NCC]
            vp = [[None] * NKC, [None] * NKC]
            oT = [[None] * NCC, [None] * NCC]
            pend = [[None] * NPAIR, [None] * NPAIR]   # P pair tiles per pair
            dn_dmas = [[], []]

            def emit_qkvT_half(b, j, which):
                """produce qT[b][j] or kT[b][j] (transposed, bf16, padded)."""
                oc = j if which == 'q' else NCC + j
                ps = psm.tile([128, W], F32, tag="m", name=f"psqk{oc}_{b}")
                for cc in range(NCC):
                    for (c0, cn) in CG_N:
                        nc.tensor.matmul(
                            ps[:, c0:c0 + cn],
                            qkvw[cc][:, oc * 128:(oc + 1) * 128],
                            xts[b][cc][:, c0:c0 + cn],
                            start=(cc == 0),
                            stop=(cc == NCC - 1),
                        )
                dst = perb.tile([128, W], BF16, tag=f"{which}T{j}",
                                name=f"{which}T{j}_{b}")
                nc.vector.tensor_copy(dst[:, 0:N], ps[:, 0:N])
                nc.vector.memset(dst[:, N:W], 0.0)
                if which == 'q':
                    qT[b][j] = dst
                else:
                    kT[b][j] = dst

            def emit_v_chunk(b, kc):
                kr = _kr(kc)
                ps = psm.tile([128, W], F32, tag="m", name=f"psv{kc}_{b}")
                for cc in range(NCC):
                    for (c0, cn) in CG_C:
                        nc.tensor.matmul(
                            ps[0:kr, c0:c0 + cn],
                            xts[b][cc][:, kc * 128:kc * 128 + kr],
                            qkvw[cc][:, 2 * C + c0:2 * C + c0 + cn],
                            start=(cc == 0),
                            stop=(cc == NCC - 1),
                        )
                t = perb.tile([128, H * (HD + 1)], BF16, tag=f"vp{kc}",
                              name=f"vp{kc}_{b}")
                v3 = t[:].rearrange("p (h e) -> p h e", e=HD + 1)
                nc.vector.tensor_copy(
                    v3[0:kr, :, 0:HD],
                    ps[0:kr, 0:C].rearrange("p (h d) -> p h d", d=HD),
                )
                nc.vector.memset(v3[0:kr, :, HD:HD + 1], 1.0)
                vp[b][kc] = t

            def emit_pass1_step(b, j, kc):
                """One kc step: S matmuls + exp + expB-multiply, both heads.

                Per-head 2-bank S psum tiles; pss bufs=2 with 2 allocs/kc
                means S(hh, kc) reuses the slot of S(hh', kc-1) whose
                consumer is one exp earlier in the ACT chain, so PE writes
                S(kc) while ACT still exps (kc-1): the chain stays packed."""
                kr = _kr(kc)
                if pend[b][j] is None:
                    pend[b][j] = [[None, None] for _ in range(NKC)]
                ebt = expbp.tile([128, W2], BF16, tag="expb",
                                 name=f"ebt{j}_{kc}_{b}")
                nc.sync.dma_start(
                    ebt[0:kr, :], expP_d[j, kc * 128:kc * 128 + kr, :]
                )
                for hh in range(2):
                    po_ = hh * 64
                    sp = pss.tile([128, W], F32, tag="s",
                                  name=f"sp{j}_{kc}_{hh}_{b}")
                    for (c0, cn) in CG_N:
                        nc.tensor.matmul(
                            sp[0:kr, c0:c0 + cn],
                            kT[b][j][po_:po_ + 64, kc * 128:kc * 128 + kr],
                            qT[b][j][po_:po_ + 64, c0:c0 + cn],
                            start=True,
                            stop=True,
                        )
                    es = flow.tile([128, W], BF16, tag="expS",
                                   name=f"es{j}_{kc}_{hh}_{b}")
                    nc.scalar.activation(
                        es[0:kr, :], sp[0:kr, 0:W],
                        mybir.ActivationFunctionType.Exp,
                    )
                    pt = ptp.tile([128, W], BF16, tag="pT",
                                  name=f"pt{j}_{kc}_{hh}_{b}")
                    nc.vector.tensor_tensor(
                        pt[0:kr, :], es[0:kr, :],
                        ebt[0:kr, hh * W:(hh + 1) * W],
                        mybir.AluOpType.mult,
                    )
                    pend[b][j][kc][hh] = pt

            ps_o_state = {}

            def emit_pass2_mms(b, j, hh, kcs):
                """A slice of the O-accumulation convoy for head 2j+hh."""
                h = 2 * j + hh
                if kcs[0] == 0:
                    ps_o_state[(b, j, hh)] = psm.tile(
                        [128, W], F32, tag="m", name=f"pso{h}_{b}")
                ps_o = ps_o_state[(b, j, hh)]
                pts = pend[b][j]
                for kc in kcs:
                    kr = _kr(kc)
                    for (c0, cn) in CG_N:
                        nc.tensor.matmul(
                            ps_o[0:HD + 1, c0:c0 + cn],
                            vp[b][kc][0:kr, h * (HD + 1):(h + 1) * (HD + 1)],
                            pts[kc][hh][0:kr, c0:c0 + cn],
                            start=(kc == 0),
                            stop=(kc == NKC - 1),
                        )

            def emit_pass2_end(b, j, hh):
                h = 2 * j + hh
                ps_o = ps_o_state.pop((b, j, hh))
                if oT[b][0] is None:
                    for cc in range(NCC):
                        oT[b][cc] = perb.tile([128, W], BF16, tag=f"oT{cc}",
                                              name=f"oT{cc}_{b}")
                nc.vector.tensor_copy(
                    oT[b][j][hh * 64:hh * 64 + 64, 0:N], ps_o[0:64, 0:N]
                )
                dn = norm.tile([65, W], BF16, tag="dn", bufs=2,
                               name=f"dn{h}_{b}")
                nc.vector.tensor_copy(dn[64:65, 0:N], ps_o[64:65, 0:N])
                drow = dall_d[b]
                dst = bass.AP(tensor=drow.tensor, offset=drow.offset + h * N,
                              ap=[[1, N]])
                dma = nc.sync.dma_start(dst, dn[64:65, 0:N])
                dn_dmas[b].append(dma)

            def emit_recip(b, rows, cols, deps, off=0):
                """reshaped reciprocal over flat dall[off : off+rows*cols]."""
                dfl = norm.tile([128, DFLN], BF16, tag="dfl", bufs=2,
                                name=f"dfl_{b}_{off}_{rows}")
                srow = dall_d[b]
                src = bass.AP(tensor=srow.tensor, offset=srow.offset + off,
                              ap=[[cols, rows], [1, cols]])
                ld = nc.sync.dma_start(dfl[0:rows, 0:cols], src)
                for di in deps:
                    add_dep_helper(ld.ins, dn_dmas[b][di].ins, sync=True,
                                   reason="dall rows before reshape load")
                dfi = norm.tile([128, DFLN], BF16, tag="dfi", bufs=2,
                                name=f"dfi_{b}_{off}_{rows}")
                with nc.allow_low_precision(
                    reason="denominators ~800; bf16 rel err 0.4% is in budget"
                ):
                    nc.vector.reciprocal(dfi[0:rows, 0:cols],
                                         dfl[0:rows, 0:cols])
                irow = dinv_d[b]
                idst = bass.AP(tensor=irow.tensor, offset=irow.offset + off,
                               ap=[[cols, rows], [1, cols]])
                return nc.sync.dma_start(idst, dfi[0:rows, 0:cols])

            def emit_norm_mults(b, ccs, sts):
                for cc in ccs:
                    dr = norm.tile([128, W], BF16, tag="drep", bufs=2,
                                   name=f"dr{cc}_{b}")
                    irow = dinv_d[b]
                    for hh in range(2):
                        h = 2 * cc + hh
                        bsrc = bass.AP(tensor=irow.tensor,
                                       offset=irow.offset + h * N,
                                       ap=[[0, 64], [1, N]])
                        bd = nc.sync.dma_start(dr[hh * 64:(hh + 1) * 64, 0:N],
                                               bsrc)
                        st = sts[hh] if isinstance(sts, (list, tuple)) else sts
                        add_dep_helper(bd.ins, st.ins, sync=True,
                                       reason="dinv store before broadcast")
                    nc.vector.tensor_tensor(
                        oT[b][cc][:, 0:N], oT[b][cc][:, 0:N], dr[:, 0:N],
                        mybir.AluOpType.mult,
                    )

            def emit_proj_chunk(b, tt, pool=None):
                ts_ = _kr(tt)
                pool, tg = pool or (psm, "m")
                ps = pool.tile([128, W], F32, tag=tg, name=f"psp{tt}_{b}")
                for cc in range(NCC):
                    for (c0, cn) in CG_C:
                        nc.tensor.matmul(
                            ps[0:ts_, c0:c0 + cn],
                            oT[b][cc][:, tt * 128:tt * 128 + ts_],
                            pw16[cc][:, c0:c0 + cn],
                            start=(cc == 0),
                            stop=(cc == NCC - 1),
                        )
                ob = outp.tile([128, C], BF16, tag="ob", name=f"ob{tt}_{b}")
                nc.vector.tensor_copy(ob[0:ts_, :], ps[0:ts_, 0:C])
                nc.sync.dma_start(
                    out_d[b, tt * 128:tt * 128 + ts_, :], ob[0:ts_, :]
                )

            # ---- filler units: one psum convoy each, slotted into the
            # kc-steps of each pair so the PE static order never has a big
            # ready-blocked block ahead of the chain-critical S matmuls ----
            def U_q(b, j, which):
                return lambda: emit_qkvT_half(b, j, which)

            def U_v(b, kc):
                return lambda: emit_v_chunk(b, kc)

            def U_p(b, tt):
                return lambda: emit_proj_chunk(b, tt)

            def U_x1():
                def f():
                    xts[1] = emit_x(1)
                return f

            UNITS = {
                (0, 0): [U_q(0, 1, 'q')], (0, 1): [U_q(0, 1, 'k')],
                (0, 2): [U_v(0, 0)], (0, 3): [U_v(0, 1)],
                (0, 4): [U_v(0, 2)], (0, 5): [U_v(0, 3)],
                (0, 6): [U_v(0, 4)],
                (1, 0): [U_v(0, 5), U_v(0, 6)],
                (1, 4): [U_q(0, 2, 'q'), U_q(0, 2, 'k')],
                (2, 0): [U_q(0, 3, 'q'), U_x1()],
                (2, 4): [U_q(0, 3, 'k'), U_q(0, 4, 'q')],
                (3, 0): [U_q(0, 4, 'k'), emit_weight_loads_proj],
                (3, 4): [U_q(0, 5, 'q'), U_q(0, 5, 'k')],
                (4, 0): [U_q(1, 0, 'q'), U_q(1, 0, 'k')],
                (4, 4): [U_q(1, 1, 'q')],
                (5, 0): [U_q(1, 1, 'k'), U_v(1, 0)],
                (5, 4): [U_v(1, 1), U_v(1, 2)],
                (6, 0): [U_v(1, 3), U_v(1, 4)],
                (6, 4): [U_v(1, 5), U_v(1, 6)],
                (7, 0): [U_q(1, 2, 'q'), U_q(1, 2, 'k')],
                (7, 4): [U_p(0, 0)],
                (8, 0): [U_q(1, 3, 'q'), U_q(1, 3, 'k')],
                (8, 4): [U_p(0, 1), U_p(0, 2)],
                (9, 0): [U_q(1, 4, 'q'), U_q(1, 4, 'k')],
                (9, 4): [U_p(0, 3), U_p(0, 4)],
                (10, 0): [U_q(1, 5, 'q'), U_q(1, 5, 'k')],
                (10, 4): [U_p(0, 5)],
                (11, 0): [U_p(0, 6)],
            }

            # ---- program ----
            # dummy exp first so the ~2.7us ACT table load happens during
            # the initial DMA wait, not before the first real exp
            warm = consts.tile([1, 8], F32, tag="warm")
            nc.vector.memset(warm[:], 0.0)
            nc.scalar.activation(warm[0:1, 4:8], warm[0:1, 0:4],
                                 mybir.ActivationFunctionType.Exp)
            emit_x0_and_weights()
            emit_qkvT_half(0, 0, 'q')
            emit_qkvT_half(0, 0, 'k')

            for p in range(2 * NPAIR):
                b, j = divmod(p, NPAIR)
                prev = p - 1
                pb, pj = divmod(prev, NPAIR) if prev >= 0 else (None, None)
                for kc in range(NKC):
                    emit_pass1_step(b, j, kc)
                    for u in UNITS.get((p, kc), []):
                        u()
                    if prev >= 0:
                        if kc == 2:
                            emit_pass2_mms(pb, pj, 0, list(range(0, 4)))
                        elif kc == 3:
                            emit_pass2_mms(pb, pj, 0, list(range(4, NKC)))
                            emit_pass2_end(pb, pj, 0)
                        elif kc == 5:
                            emit_pass2_mms(pb, pj, 1, list(range(0, 4)))
                        elif kc == 6:
                            emit_pass2_mms(pb, pj, 1, list(range(4, NKC)))
                            emit_pass2_end(pb, pj, 1)
                    if p == 2 * NPAIR - 1 and kc == 6:
                        # early start on the last pair's own hh0 convoy
                        emit_pass2_mms(1, NPAIR - 1, 0, list(range(0, 4)))
                if prev >= 0 and (pb, pj) == (0, NPAIR - 1):
                    st0 = emit_recip(0, 128, DFLN, range(12))
                    emit_norm_mults(0, range(NCC), st0)
                if p == 2 * NPAIR - 1:
                    # partial norm for batch 1 (heads 0-9) so only cc=5's
                    # chain remains after the last pair
                    st1a = emit_recip(1, 127, 62, range(10))
                    emit_norm_mults(1, range(NCC - 1), st1a)

            emit_pass2_mms(1, NPAIR - 1, 0, list(range(4, NKC)))
            emit_pass2_end(1, NPAIR - 1, 0)
            # head-10 reciprocal chain overlaps the hh1 convoy
            st2a = emit_recip(1, 99, 8, [10], off=10 * N)
            emit_pass2_mms(1, NPAIR - 1, 1, list(range(NKC)))
            emit_pass2_end(1, NPAIR - 1, 1)
            st2b = emit_recip(1, 99, 8, [11], off=11 * N)
            emit_norm_mults(1, [NCC - 1], (st2a, st2b))
            # epilogue: S psum slots are free, so alternate pools for a
            # deeper rotation (back-to-back convoys instead of evac-gated)
            for tt in range(NKC):
                emit_proj_chunk(1, tt,
                                pool=(pss, "s") if tt % 2 else (psm, "m"))

    nc.compile()
    return nc


def _relative_position_index():
    coords = np.stack(np.meshgrid(np.arange(WX), np.arange(WY), indexing="ij"))
    cf = coords.reshape(2, -1)
    rel = cf[:, :, None] - cf[:, None, :]
    rel = rel.transpose(1, 2, 0).astype(np.int64)
    rel[:, :, 0] += WX - 1
    rel[:, :, 1] += WY - 1
    rel[:, :, 0] *= 2 * WY - 1
    return rel.sum(-1)  # [L, L]


def _host_prep(x, qkv_w, proj_w, proj_b, rel_table, g2l, g2g):
    x = np.asarray(x, np.float32)
    qkv_w = np.asarray(qkv_w, np.float32)
    proj_w = np.asarray(proj_w, np.float32)
    rel_table = np.asarray(rel_table, np.float32)
    g2l = np.asarray(g2l, np.float32)
    g2g = np.asarray(g2g, np.float32)

    bf16 = ml_dtypes.bfloat16
    xT = np.ascontiguousarray(x.transpose(0, 2, 1)).astype(bf16)   # [B, C, N]
    qkv_wT = np.ascontiguousarray(qkv_w.T).copy()                  # [C, 3C]
    qkv_wT[:, :C] *= SCALE                                         # fold q scale
    qkv_wT = qkv_wT.astype(bf16)
    proj_wT = np.ascontiguousarray(proj_w.T).astype(bf16)          # [C, C]

    # expB[h, k, q] = exp(bias[h, q, k]); exp applied at table granularity,
    # then expanded by the constant-index relative-position gather.
    # Pair layout for the merged multiply: expP[j, k, hh*W + q].
    ridx = _relative_position_index()
    et = np.exp(rel_table)                                         # [3025, H]
    eg2l = np.exp(g2l)                                             # [2, H, 1]
    eg2g = np.exp(g2g)                                             # [H, 1, 1]
    expB = np.zeros((H, N, W), np.float32)
    expB[:, 1:, 1:N] = et[ridx].transpose(2, 1, 0)                 # [H, k, q]
    expB[:, 0, 0] = eg2g[:, 0, 0]
    expB[:, 1:, 0] = eg2l[0][:, 0][None, :].T                      # global query
    expB[:, 0, 1:N] = eg2l[1][:, 0][:, None]                       # global key
    expP = np.ascontiguousarray(
        expB.reshape(NPAIR, 2, N, W).transpose(0, 2, 1, 3).reshape(NPAIR, N, W2)
    ).astype(bf16)

    in_maps = []
    for i in range(N_CORES):
        in_maps.append({
            "xT": xT[i * B_LOC:(i + 1) * B_LOC],
            "qkv_wT": qkv_wT,
            "proj_wT": proj_wT,
            "expP": expP,
        })
    return in_maps


_NC = None


def get_nc():
    global _NC
    if _NC is None:
        _NC = build_nc()
    return _NC


def kernel(x, qkv_w, proj_w, proj_b, rel_table, g2l, g2g):
    in_maps = _host_prep(x, qkv_w, proj_w, proj_b, rel_table, g2l, g2g)
    nc = get_nc()
    res = run_bass_kernel_spmd(nc, in_maps, core_ids=list(range(N_CORES)))
    out = np.concatenate([res.results[i]["out"] for i in range(N_CORES)], axis=0)
    return out.astype(np.float32) + np.asarray(proj_b, np.float32)


# revision 32
# speedup vs baseline: 1.0617x; 1.0617x over previous
"""Trainium2 Bass kernel for windowed/global sparse attention (Swin-style
relative-position bias + 1 global token), data-parallel over batch on 8 cores.

Shapes: B=16, N=785 (1 global + 28x28 local), C=768, H=12 heads, d=64.

v2 design notes (from trace analysis of v1, which was ACT-bound at 97.7%):
  - Head-PAIR-merged attention inner loop: both heads of a pair write one
    4-bank PSUM tile ([128, 2048] f32, hh1 at column 1024), so exp is a
    SINGLE ACTIVATE per (pair, kc) with a 3D access pattern (FD=1572),
    halving ScalarE instruction count and its ~470ns/instr overhead.
    The expB multiply likewise becomes a single DVE tensor_tensor on a
    host-pre-paired [128, 1572] bf16 tile.
  - S^T matmuls for the two heads run concurrently via PE row-tiling
    (tile_position auto-derived from base partitions 0/64, K=64 each).
  - Unified 12-slot pair pipeline across both batches with fillers
    (qkv chunks, V, x loads, proj of the previous batch) emitted BETWEEN
    pair emissions so the PSUM rotor interleaves them (v1's proj0 slid to
    the end of the program because its psum allocations sat behind all of
    batch-1's attention in rotor order, running ~55us at cold clock).
  - Denominators: ones-column in V (65th lhsT column) as v1; the
    reciprocal is computed on a [128, 74]-reshaped copy (DMA round trip
    through DRAM) instead of [12, 785] - DVE reciprocal is 8 cyc/elem
    per lane, so the reshape turns a 6.5us op into ~0.6us.
  - proj bias is added on the HOST (free), device emits bf16 output.
  - PSUM budget (8 banks): S-pair tile 4 + O convoy 2 + filler 2.
"""

import numpy as np
import ml_dtypes

import concourse.bass as bass
import concourse.bacc as bacc
import concourse.tile as tile
from concourse.tile import add_dep_helper
from concourse import mybir
from concourse.bass_utils import run_bass_kernel_spmd

F32 = mybir.dt.float32
BF16 = mybir.dt.bfloat16

WX = WY = 28
NGLO = 1
H = 12
L = WX * WY            # 784
N = NGLO + L           # 785
C = 768
HD = C // H            # 64
SCALE = HD ** -0.5
B = 16
N_CORES = 8
B_LOC = B // N_CORES   # 2
NCC = C // 128         # 6 contraction chunks
NKC = (N + 127) // 128  # 7 key/token chunks (last = 17 rows)
NPAIR = H // 2         # 6 head pairs
W = 786                # padded free width for N-sized tiles (even, 4B-aligned)
W2 = 2 * W             # pair-merged free width (1572)
SP_OFF = 1024          # hh1 column offset in the S-pair psum tile (bank 2)
DFLN = 74              # 128*74 = 9472 >= 12*785 reshaped reciprocal tile

CG_N = [(0, 512), (512, 274)]
CG_C = [(0, 512), (512, 256)]


def _kr(kc):
    return min(128, N - kc * 128)


def build_nc():
    nc = bacc.Bacc(None, target_bir_lowering=False)

    xT_d = nc.dram_tensor("xT", [B_LOC, C, N], BF16, kind="ExternalInput")
    qkvwT_d = nc.dram_tensor("qkv_wT", [C, 3 * C], BF16, kind="ExternalInput")
    pwT_d = nc.dram_tensor("proj_wT", [C, C], BF16, kind="ExternalInput")
    expP_d = nc.dram_tensor("expP", [NPAIR, N, W2], BF16, kind="ExternalInput")
    out_d = nc.dram_tensor("out", [B_LOC, N, C], BF16, kind="ExternalOutput")
    dall_d = nc.dram_tensor("dall_scratch", [B_LOC, 128 * DFLN], BF16)
    dinv_d = nc.dram_tensor("dinv_scratch", [B_LOC, 128 * DFLN], BF16)

    with tile.TileContext(nc) as tc:
        with (
            tc.tile_pool(name="consts", bufs=1) as consts,
            tc.tile_pool(name="perb", bufs=2) as perb,
            tc.tile_pool(name="expbp", bufs=6) as expbp,
            tc.tile_pool(name="flow", bufs=6) as flow,
            tc.tile_pool(name="ptp", bufs=20) as ptp,
            tc.tile_pool(name="norm", bufs=1) as norm,
            tc.tile_pool(name="outp", bufs=2) as outp,
            tc.tile_pool(name="pss", bufs=2, space=bass.MemorySpace.PSUM) as pss,
            tc.tile_pool(name="psm", bufs=2, space=bass.MemorySpace.PSUM) as psm,
        ):
            # ---- resident weights (bf16) ----
            qkvw = [consts.tile([128, 3 * C], BF16, tag=f"qkvw{cc}",
                                name=f"qkvw{cc}") for cc in range(NCC)]
            pw16 = [consts.tile([128, C], BF16, tag=f"pw{cc}",
                                name=f"pw{cc}") for cc in range(NCC)]

            def emit_weight_loads_proj():
                for cc in range(NCC):
                    nc.gpsimd.dma_start(
                        pw16[cc][:], pwT_d[cc * 128:(cc + 1) * 128, :]
                    )

            def emit_x(b):
                xts = []
                for cc in range(NCC):
                    t = perb.tile([128, W], BF16, tag=f"xt{cc}", name=f"xt{cc}_{b}")
                    nc.sync.dma_start(
                        t[:, 0:N], xT_d[b, cc * 128:(cc + 1) * 128, :]
                    )
                    nc.vector.memset(t[:, N:W], 0.0)
                    xts.append(t)
                return xts

            def emit_x0_and_weights():
                # The q0/k0 weight column slices and x0 are prefetched first
                # (~1.6MB, one 3D-AP DMA per cc to keep the Sync issue queue
                # short) so the first qkvT convoy -- and with it the whole
                # exp chain -- starts ~15us earlier than waiting for the
                # full 4.7MB load.  The bulk weight remainder goes through
                # the GpSimd (SWDGE) queue, off the Sync issue path.
                xts[0] = []
                for cc in range(NCC):
                    t = perb.tile([128, W], BF16, tag=f"xt{cc}",
                                  name=f"xt{cc}_0")
                    dst3 = qkvw[cc][:].rearrange("p (a c) -> p a c", c=C)
                    src3 = qkvwT_d[cc * 128:(cc + 1) * 128, :].rearrange(
                        "p (a c) -> p a c", c=C)
                    nc.sync.dma_start(dst3[:, 0:2, 0:128], src3[:, 0:2, 0:128])
                    nc.sync.dma_start(
                        t[:, 0:N], xT_d[0, cc * 128:(cc + 1) * 128, :]
                    )
                    nc.vector.memset(t[:, N:W], 0.0)
                    xts[0].append(t)
                for cc in range(NCC):
                    d3 = qkvw[cc][:].rearrange("p (a c) -> p a c", c=C)
                    s3 = qkvwT_d[cc * 128:(cc + 1) * 128, :].rearrange(
                        "p (a c) -> p a c", c=C)
                    nc.gpsimd.dma_start(d3[:, 0:2, 128:C], s3[:, 0:2, 128:C])
                    nc.gpsimd.dma_start(
                        qkvw[cc][:, 2 * C:3 * C],
                        qkvwT_d[cc * 128:(cc + 1) * 128, 2 * C:3 * C],
                    )

            # per-batch state
            xts = [None, None]
            qT = [[None] * NCC, [None] * NCC]
            kT = [[None] * NCC, [None] * NCC]
            vp = [[None] * NKC, [None] * NKC]
            oT = [[None] * NCC, [None] * NCC]
            pend = [[None] * NPAIR, [None] * NPAIR]   # P pair tiles per pair
            dn_dmas = [[], []]

            def emit_qkvT_half(b, j, which):
                """produce qT[b][j] or kT[b][j] (transposed, bf16, padded)."""
                oc = j if which == 'q' else NCC + j
                ps = psm.tile([128, W], F32, tag="m", name=f"psqk{oc}_{b}")
                for cc in range(NCC):
                    for (c0, cn) in CG_N:
                        nc.tensor.matmul(
                            ps[:, c0:c0 + cn],
                            qkvw[cc][:, oc * 128:(oc + 1) * 128],
                            xts[b][cc][:, c0:c0 + cn],
                            start=(cc == 0),
                            stop=(cc == NCC - 1),
                        )
                dst = perb.tile([128, W], BF16, tag=f"{which}T{j}",
                                name=f"{which}T{j}_{b}")
                nc.vector.tensor_copy(dst[:, 0:N], ps[:, 0:N])
                nc.vector.memset(dst[:, N:W], 0.0)
                if which == 'q':
                    qT[b][j] = dst
                else:
                    kT[b][j] = dst

            def emit_v_chunk(b, kc):
                kr = _kr(kc)
                ps = psm.tile([128, W], F32, tag="m", name=f"psv{kc}_{b}")
                for cc in range(NCC):
                    for (c0, cn) in CG_C:
                        nc.tensor.matmul(
                            ps[0:kr, c0:c0 + cn],
                            xts[b][cc][:, kc * 128:kc * 128 + kr],
                            qkvw[cc][:, 2 * C + c0:2 * C + c0 + cn],
                            start=(cc == 0),
                            stop=(cc == NCC - 1),
                        )
                t = perb.tile([128, H * (HD + 1)], BF16, tag=f"vp{kc}",
                              name=f"vp{kc}_{b}")
                v3 = t[:].rearrange("p (h e) -> p h e", e=HD + 1)
                nc.vector.tensor_copy(
                    v3[0:kr, :, 0:HD],
                    ps[0:kr, 0:C].rearrange("p (h d) -> p h d", d=HD),
                )
                nc.vector.memset(v3[0:kr, :, HD:HD + 1], 1.0)
                vp[b][kc] = t

            def emit_pass1_step(b, j, kc):
                """One kc step: S matmuls + exp + expB-multiply, both heads.

                Per-head 2-bank S psum tiles; pss bufs=2 with 2 allocs/kc
                means S(hh, kc) reuses the slot of S(hh', kc-1) whose
                consumer is one exp earlier in the ACT chain, so PE writes
                S(kc) while ACT still exps (kc-1): the chain stays packed."""
                kr = _kr(kc)
                if pend[b][j] is None:
                    pend[b][j] = [[None, None] for _ in range(NKC)]
                ebt = expbp.tile([128, W2], BF16, tag="expb",
                                 name=f"ebt{j}_{kc}_{b}")
                nc.sync.dma_start(
                    ebt[0:kr, :], expP_d[j, kc * 128:kc * 128 + kr, :]
                )
                for hh in range(2):
                    po_ = hh * 64
                    sp = pss.tile([128, W], F32, tag="s",
                                  name=f"sp{j}_{kc}_{hh}_{b}")
                    for (c0, cn) in CG_N:
                        nc.tensor.matmul(
                            sp[0:kr, c0:c0 + cn],
                            kT[b][j][po_:po_ + 64, kc * 128:kc * 128 + kr],
                            qT[b][j][po_:po_ + 64, c0:c0 + cn],
                            start=True,
                            stop=True,
                        )
                    es = flow.tile([128, W], BF16, tag="expS",
                                   name=f"es{j}_{kc}_{hh}_{b}")
                    nc.scalar.activation(
                        es[0:kr, :], sp[0:kr, 0:W],
                        mybir.ActivationFunctionType.Exp,
                    )
                    pt = ptp.tile([128, W], BF16, tag="pT",
                                  name=f"pt{j}_{kc}_{hh}_{b}")
                    nc.vector.tensor_tensor(
                        pt[0:kr, :], es[0:kr, :],
                        ebt[0:kr, hh * W:(hh + 1) * W],
                        mybir.AluOpType.mult,
                    )
                    pend[b][j][kc][hh] = pt

            ps_o_state = {}

            def emit_pass2_mms(b, j, hh, kcs):
                """A slice of the O-accumulation convoy for head 2j+hh."""
                h = 2 * j + hh
                if kcs[0] == 0:
                    ps_o_state[(b, j, hh)] = psm.tile(
                        [128, W], F32, tag="m", name=f"pso{h}_{b}")
                ps_o = ps_o_state[(b, j, hh)]
                pts = pend[b][j]
                for kc in kcs:
                    kr = _kr(kc)
                    for (c0, cn) in CG_N:
                        nc.tensor.matmul(
                            ps_o[0:HD + 1, c0:c0 + cn],
                            vp[b][kc][0:kr, h * (HD + 1):(h + 1) * (HD + 1)],
                            pts[kc][hh][0:kr, c0:c0 + cn],
                            start=(kc == 0),
                            stop=(kc == NKC - 1),
                        )

            def emit_pass2_end(b, j, hh):
                h = 2 * j + hh
                ps_o = ps_o_state.pop((b, j, hh))
                if oT[b][0] is None:
                    for cc in range(NCC):
                        oT[b][cc] = perb.tile([128, W], BF16, tag=f"oT{cc}",
                                              name=f"oT{cc}_{b}")
                nc.vector.tensor_copy(
                    oT[b][j][hh * 64:hh * 64 + 64, 0:N], ps_o[0:64, 0:N]
                )
                dn = norm.tile([65, W], BF16, tag="dn", bufs=2,
                               name=f"dn{h}_{b}")
                nc.vector.tensor_copy(dn[64:65, 0:N], ps_o[64:65, 0:N])
                drow = dall_d[b]
                dst = bass.AP(tensor=drow.tensor, offset=drow.offset + h * N,
                              ap=[[1, N]])
                dma = nc.sync.dma_start(dst, dn[64:65, 0:N])
                dn_dmas[b].append(dma)

            def emit_recip(b, rows, cols, deps, off=0):
                """reshaped reciprocal over flat dall[off : off+rows*cols]."""
                dfl = norm.tile([128, DFLN], BF16, tag="dfl", bufs=2,
                                name=f"dfl_{b}_{off}_{rows}")
                srow = dall_d[b]
                src = bass.AP(tensor=srow.tensor, offset=srow.offset + off,
                              ap=[[cols, rows], [1, cols]])
                ld = nc.sync.dma_start(dfl[0:rows, 0:cols], src)
                for di in deps:
                    add_dep_helper(ld.ins, dn_dmas[b][di].ins, sync=True,
                                   reason="dall rows before reshape load")
                dfi = norm.tile([128, DFLN], BF16, tag="dfi", bufs=2,
                                name=f"dfi_{b}_{off}_{rows}")
                with nc.allow_low_precision(
                    reason="denominators ~800; bf16 rel err 0.4% is in budget"
                ):
                    nc.vector.reciprocal(dfi[0:rows, 0:cols],
                                         dfl[0:rows, 0:cols])
                irow = dinv_d[b]
                idst = bass.AP(tensor=irow.tensor, offset=irow.offset + off,
                               ap=[[cols, rows], [1, cols]])
                return nc.sync.dma_start(idst, dfi[0:rows, 0:cols])

            def emit_norm_mults(b, ccs, sts):
                for cc in ccs:
                    dr = norm.tile([128, W], BF16, tag="drep", bufs=2,
                                   name=f"dr{cc}_{b}")
                    irow = dinv_d[b]
                    for hh in range(2):
                        h = 2 * cc + hh
                        bsrc = bass.AP(tensor=irow.tensor,
                                       offset=irow.offset + h * N,
                                       ap=[[0, 64], [1, N]])
                        bd = nc.sync.dma_start(dr[hh * 64:(hh + 1) * 64, 0:N],
                                               bsrc)
                        st = sts[hh] if isinstance(sts, (list, tuple)) else sts
                        add_dep_helper(bd.ins, st.ins, sync=True,
                                       reason="dinv store before broadcast")
                    nc.vector.tensor_tensor(
                        oT[b][cc][:, 0:N], oT[b][cc][:, 0:N], dr[:, 0:N],
                        mybir.AluOpType.mult,
                    )

            def emit_proj_chunk(b, tt, pool=None):
                ts_ = _kr(tt)
                pool, tg = pool or (psm, "m")
                ps = pool.tile([128, W], F32, tag=tg, name=f"psp{tt}_{b}")
                for cc in range(NCC):
                    for (c0, cn) in CG_C:
                        nc.tensor.matmul(
                            ps[0:ts_, c0:c0 + cn],
                            oT[b][cc][:, tt * 128:tt * 128 + ts_],
                            pw16[cc][:, c0:c0 + cn],
                            start=(cc == 0),
                            stop=(cc == NCC - 1),
                        )
                ob = outp.tile([128, C], BF16, tag="ob", name=f"ob{tt}_{b}")
                nc.vector.tensor_copy(ob[0:ts_, :], ps[0:ts_, 0:C])
                nc.sync.dma_start(
                    out_d[b, tt * 128:tt * 128 + ts_, :], ob[0:ts_, :]
                )

            # ---- filler units: one psum convoy each, slotted into the
            # kc-steps of each pair so the PE static order never has a big
            # ready-blocked block ahead of the chain-critical S matmuls ----
            def U_q(b, j, which):
                return lambda: emit_qkvT_half(b, j, which)

            def U_v(b, kc):
                return lambda: emit_v_chunk(b, kc)

            def U_p(b, tt):
                return lambda: emit_proj_chunk(b, tt)

            def U_x1():
                def f():
                    xts[1] = emit_x(1)
                return f

            # strictly ONE psum-convoy unit per slot: two units back-to-back
            # put ~4.4us of ready PE work ahead of the next S matmuls in the
            # engine's static order and stall the exp chain (measured v5)
            UNITS = {
                (0, 0): [U_q(0, 1, 'q')], (0, 1): [U_q(0, 1, 'k')],
                (0, 2): [U_v(0, 0)], (0, 3): [U_v(0, 1)],
                (0, 4): [U_v(0, 2)], (0, 5): [U_v(0, 3)],
                (0, 6): [U_v(0, 4)],
                (1, 0): [U_v(0, 5)], (1, 2): [U_v(0, 6)],
                (1, 4): [U_q(0, 2, 'q')], (1, 6): [U_q(0, 2, 'k')],
                (2, 0): [U_x1()],
                (2, 2): [U_q(0, 3, 'q')], (2, 4): [U_q(0, 3, 'k')],
                (3, 0): [U_q(0, 4, 'q'), emit_weight_loads_proj],
                (3, 2): [U_q(0, 4, 'k')], (3, 4): [U_q(0, 5, 'q')],
                (4, 0): [U_q(0, 5, 'k')], (4, 2): [U_q(1, 0, 'q')],
                (4, 4): [U_q(1, 0, 'k')],
                (5, 0): [U_q(1, 1, 'q')], (5, 2): [U_q(1, 1, 'k')],
                (5, 4): [U_v(1, 0)],
                (6, 0): [U_v(1, 1)], (6, 2): [U_v(1, 2)],
                (6, 4): [U_v(1, 3)], (6, 6): [U_v(1, 4)],
                (7, 0): [U_v(1, 5)], (7, 2): [U_v(1, 6)],
                (7, 4): [U_q(1, 2, 'q')], (7, 6): [U_q(1, 2, 'k')],
                (8, 0): [U_q(1, 3, 'q')], (8, 2): [U_q(1, 3, 'k')],
                (9, 0): [U_q(1, 4, 'q')], (9, 2): [U_q(1, 4, 'k')],
                (9, 4): [U_p(0, 0)],
                (10, 0): [U_q(1, 5, 'q')], (10, 2): [U_q(1, 5, 'k')],
                (10, 4): [U_p(0, 1)],
                (11, 0): [U_p(0, 2)], (11, 2): [U_p(0, 3)],
                (11, 4): [U_p(0, 4)],
            }

            # ---- program ----
            # dummy exp first so the ~2.7us ACT table load happens during
            # the initial DMA wait, not before the first real exp
            warm = consts.tile([1, 8], F32, tag="warm")
            nc.vector.memset(warm[:], 0.0)
            nc.scalar.activation(warm[0:1, 4:8], warm[0:1, 0:4],
                                 mybir.ActivationFunctionType.Exp)
            emit_x0_and_weights()
            emit_qkvT_half(0, 0, 'q')
            emit_qkvT_half(0, 0, 'k')

            for p in range(2 * NPAIR):
                b, j = divmod(p, NPAIR)
                prev = p - 1
                pb, pj = divmod(prev, NPAIR) if prev >= 0 else (None, None)
                for kc in range(NKC):
                    emit_pass1_step(b, j, kc)
                    for u in UNITS.get((p, kc), []):
                        u()
                    if prev >= 0:
                        if kc == 1:
                            emit_pass2_mms(pb, pj, 0, list(range(0, 4)))
                        elif kc == 3:
                            emit_pass2_mms(pb, pj, 0, list(range(4, NKC)))
                            emit_pass2_end(pb, pj, 0)
                        elif kc == 5:
                            emit_pass2_mms(pb, pj, 1, list(range(0, 4)))
                        elif kc == 6:
                            emit_pass2_mms(pb, pj, 1, list(range(4, NKC)))
                            emit_pass2_end(pb, pj, 1)
                    if p == 2 * NPAIR - 1 and kc == 6:
                        # early start on the last pair's own hh0 convoy
                        emit_pass2_mms(1, NPAIR - 1, 0, list(range(0, 4)))
                if prev >= 0 and (pb, pj) == (0, NPAIR - 1):
                    st0 = emit_recip(0, 128, DFLN, range(12))
                    emit_norm_mults(0, range(NCC), st0)
                if p == 2 * NPAIR - 1:
                    # partial norm for batch 1 (heads 0-9) so only cc=5's
                    # chain remains after the last pair
                    st1a = emit_recip(1, 127, 62, range(10))
                    emit_norm_mults(1, range(NCC - 1), st1a)

            emit_pass2_mms(1, NPAIR - 1, 0, list(range(4, NKC)))
            emit_pass2_end(1, NPAIR - 1, 0)
            # head-10 reciprocal chain overlaps the hh1 convoy
            st2a = emit_recip(1, 99, 8, [10], off=10 * N)
            emit_pass2_mms(1, NPAIR - 1, 1, list(range(NKC)))
            emit_pass2_end(1, NPAIR - 1, 1)
            st2b = emit_recip(1, 99, 8, [11], off=11 * N)
            # independent proj0 leftovers give PE work under the norm chain
            emit_proj_chunk(0, 5)
            emit_proj_chunk(0, 6)
            emit_norm_mults(1, [NCC - 1], (st2a, st2b))
            # epilogue: S psum slots are free, so alternate pools for a
            # deeper rotation (back-to-back convoys instead of evac-gated)
            for tt in range(NKC):
                emit_proj_chunk(1, tt,
                                pool=(pss, "s") if tt % 2 else (psm, "m"))

    nc.compile()
    return nc


def _relative_position_index():
    coords = np.stack(np.meshgrid(np.arange(WX), np.arange(WY), indexing="ij"))
    cf = coords.reshape(2, -1)
    rel = cf[:, :, None] - cf[:, None, :]
    rel = rel.transpose(1, 2, 0).astype(np.int64)
    rel[:, :, 0] += WX - 1
    rel[:, :, 1] += WY - 1
    rel[:, :, 0] *= 2 * WY - 1
    return rel.sum(-1)  # [L, L]


def _host_prep(x, qkv_w, proj_w, proj_b, rel_table, g2l, g2g):
    x = np.asarray(x, np.float32)
    qkv_w = np.asarray(qkv_w, np.float32)
    proj_w = np.asarray(proj_w, np.float32)
    rel_table = np.asarray(rel_table, np.float32)
    g2l = np.asarray(g2l, np.float32)
    g2g = np.asarray(g2g, np.float32)

    bf16 = ml_dtypes.bfloat16
    xT = np.ascontiguousarray(x.transpose(0, 2, 1)).astype(bf16)   # [B, C, N]
    qkv_wT = np.ascontiguousarray(qkv_w.T).copy()                  # [C, 3C]
    qkv_wT[:, :C] *= SCALE                                         # fold q scale
    qkv_wT = qkv_wT.astype(bf16)
    proj_wT = np.ascontiguousarray(proj_w.T).astype(bf16)          # [C, C]

    # expB[h, k, q] = exp(bias[h, q, k]); exp applied at table granularity,
    # then expanded by the constant-index relative-position gather.
    # Pair layout for the merged multiply: expP[j, k, hh*W + q].
    ridx = _relative_position_index()
    et = np.exp(rel_table)                                         # [3025, H]
    eg2l = np.exp(g2l)                                             # [2, H, 1]
    eg2g = np.exp(g2g)                                             # [H, 1, 1]
    expB = np.zeros((H, N, W), np.float32)
    expB[:, 1:, 1:N] = et[ridx].transpose(2, 1, 0)                 # [H, k, q]
    expB[:, 0, 0] = eg2g[:, 0, 0]
    expB[:, 1:, 0] = eg2l[0][:, 0][None, :].T                      # global query
    expB[:, 0, 1:N] = eg2l[1][:, 0][:, None]                       # global key
    expP = np.ascontiguousarray(
        expB.reshape(NPAIR, 2, N, W).transpose(0, 2, 1, 3).reshape(NPAIR, N, W2)
    ).astype(bf16)

    in_maps = []
    for i in range(N_CORES):
        in_maps.append({
            "xT": xT[i * B_LOC:(i + 1) * B_LOC],
            "qkv_wT": qkv_wT,
            "proj_wT": proj_wT,
            "expP": expP,
        })
    return in_maps


_NC = None


def get_nc():
    global _NC
    if _NC is None:
        _NC = build_nc()
    return _NC


def kernel(x, qkv_w, proj_w, proj_b, rel_table, g2l, g2g):
    in_maps = _host_prep(x, qkv_w, proj_w, proj_b, rel_table, g2l, g2g)
    nc = get_nc()
    res = run_bass_kernel_spmd(nc, in_maps, core_ids=list(range(N_CORES)))
    out = np.concatenate([res.results[i]["out"] for i in range(N_CORES)], axis=0)
    return out.astype(np.float32) + np.asarray(proj_b, np.float32)
